# revision 20
# baseline (speedup 1.0000x reference)
"""Laplacian pyramid (5 levels) for (16,3,1024,1024) fp32, data-parallel on 8 NeuronCores.

Per core: 2 images x 3 channels = 6 planes of 1024x1024, processed as 3 plane
pairs packed side-by-side along the SBUF free dimension.
All convolutions run on the TensorEngine as accumulated banded matmuls
(float32r operands stream at full rate):
  - reduce:  5 matmuls (one per horizontal tap, scale folded into lhsT),
             vertical 5-tap stride-2 band in lhsT, horizontal stride-2 via
             strided rhs access patterns.
  - expand:  polyphase; fine output rows interleaved in lhsT (even rows use
             3-tap, odd rows 2-tap vertical filters); horizontal taps as
             accumulated matmuls per output-column parity.
lp = g - expand(g') via VectorEngine tensor_tensor subtract reading PSUM.
Gaussian levels g1..g4 stay in SBUF on a unified 124-stride row-window grid:
window j of a level holds rows [124j-1, 124j+127) so one window set serves
the level's reduce rhs, the coarser expand rhs, and the subtract minuend.
Windows are assembled once per level with partition-shifting SBUF->SBUF DMA,
alternating between the HWDGE (sync) and SWDGE (gpsimd) paths.
"""

import os
import sys
from contextlib import ExitStack

import numpy as np

for _p in ("/opt/trn_rl_repo",):
    if os.path.isdir(_p) and _p not in sys.path:
        sys.path.insert(0, _p)

import concourse.bass as bass
import concourse.mybir as mybir
from concourse import bacc
import concourse.tile as tile
from concourse.bass_utils import run_bass_kernel_spmd

F32 = mybir.dt.float32
F32R = mybir.dt.float32r

_H5 = np.array([1.0, 4.0, 6.0, 4.0, 1.0]) / 16.0
_VE = 2.0 * _H5[0::2]  # [0.125, 0.75, 0.125] even-row/col expand taps
_VO = 2.0 * _H5[1::2]  # [0.5, 0.5]           odd-row/col expand taps
_RTAPS = (0, 4, 1, 3, 2)  # reduce tap order: equal-scale taps adjacent
_QTAPS = ((0, (0, 2, 1)), (1, (0, 1)))  # (col parity, horizontal taps)


def _grid(Hl):
    """Unified row-window grid: window j = rows [max(0,124j-1), 124j+127)."""
    ws = []
    j = 0
    while True:
        a = max(0, 124 * j - 1)
        e = min(Hl, 124 * j + 127)
        ws.append((a, e - a))
        if e >= Hl:
            return ws
        j += 1


def _blocks(Hl):
    """Reduce blocking on the grid: window j -> output rows [lo, lo+M)."""
    Ho = Hl // 2
    out = []
    for j, (a, K) in enumerate(_grid(Hl)):
        lo = 0 if j == 0 else 62 * j + 1
        hi = min(Ho, 62 * (j + 1) + 1)
        out.append((a, K, lo, hi - lo))
    return out


def _reduce_lhsT(Hl, r0, K, m0, M, scale):
    B = np.zeros((K, M), np.float64)
    for jj in range(M):
        for t in range(5):
            r = min(max(2 * (m0 + jj) - 2 + t, 0), Hl - 1) - r0
            assert 0 <= r < K
            B[r, jj] += _H5[t]
    return (scale * B).astype(np.float32)


def _expand_lhsT(Hc, r0, Kf, c0, Kc, scale):
    """Vertical expand band: coarse rows [c0, c0+Kc) -> fine rows [r0, r0+Kf)."""
    E = np.zeros((Kc, Kf), np.float64)
    for jj in range(Kf):
        c = (r0 + jj) // 2
        if (r0 + jj) % 2 == 0:
            for t in range(3):
                cc = min(max(c - 1 + t, 0), Hc - 1) - c0
                assert 0 <= cc < Kc
                E[cc, jj] += _VE[t]
        else:
            for t in range(2):
                cc = min(c + t, Hc - 1) - c0
                assert 0 <= cc < Kc
                E[cc, jj] += _VO[t]
    return (scale * E).astype(np.float32)


def _all_weights(H, levels):
    """Every lhsT matrix the program needs, deduped, packed into one array."""
    mats = {}  # content key -> (arr, col offset)
    off = [0]

    def reg(arr):
        key = (arr.shape, arr.tobytes())
        if key not in mats:
            mats[key] = (arr, off[0])
            off[0] += arr.shape[1]
        return key

    slot = {}
    for l in range(levels - 1):
        Hl = H >> l
        cg = _grid(Hl // 2)
        for j, (r0, K, m0, M) in enumerate(_blocks(Hl)):
            for d in _RTAPS:
                slot[(l, j, "r", d)] = reg(_reduce_lhsT(Hl, r0, K, m0, M, _H5[d]))
            c0, Kc = cg[j // 2]
            for q, taps in _QTAPS:
                hv = _VE if q == 0 else _VO
                for d in taps:
                    slot[(l, j, q, d)] = reg(
                        _expand_lhsT(Hl // 2, r0, K, c0, Kc, hv[d])
                    )
    big = np.zeros((128, off[0]), np.float32)
    for arr, o in mats.values():
        big[: arr.shape[0], o : o + arr.shape[1]] = arr
    slices = {k: (mats[key][1], mats[key][0].shape) for k, key in slot.items()}
    return big, slices


def build_nc(H=1024, planes=6, levels=5, repeat=1, parts="all"):
    do_pe = parts in ("all", "nodve")
    do_dve = parts in ("all", "nope")
    do_act = parts != "dma"
    assert planes % 2 == 0
    nc = bacc.Bacc("TRN2", target_bir_lowering=False, debug=False)
    x = nc.dram_tensor("x", [planes * H, H], F32, kind="ExternalInput")
    lps = [
        nc.dram_tensor(f"lp{l}", [planes * (H >> l), H >> l], F32, kind="ExternalOutput")
        for l in range(levels)
    ]
    xr = x[:, :].rearrange("(p h) w -> h p w", h=H)
    lprs = [lp[:, :].rearrange("(p h) w -> h p w", h=H >> l) for l, lp in enumerate(lps)]

    bigw, wsl = _all_weights(H, levels)
    # window bufs per level: level-(l+1) windows live from creation (during
    # level l) until consumed by level l+1's expand; keep the whole set + 1.
    nwin = [len(_grid(H >> l)) for l in range(levels)]
    wbufs = [6] + [n + 1 for n in nwin[1:]]

    with tile.TileContext(nc) as tc, ExitStack() as ctx:
        wpool = ctx.enter_context(tc.tile_pool(name="wts", bufs=1))
        spool = ctx.enter_context(tc.tile_pool(name="sbuf", bufs=2))
        ppool = ctx.enter_context(tc.tile_pool(name="psum", bufs=2, space="PSUM"))

        wconst = nc.inline_tensor(np.ascontiguousarray(bigw), name="bigw")
        wtile = wpool.tile(list(bigw.shape), F32, tag="bigw", bufs=1)
        nc.sync.dma_start(wtile[:].bitcast(F32R), wconst[:, :].bitcast(F32R))

        def wt(key):
            o, (K, M) = wsl[key]
            return wtile[:K, o : o + M].bitcast(F32R)

        def asm_engine():
            return nc.gpsimd

        def pad_fill(t, K, wpad):
            # replicate cols [2, w+2) outward into 2-wide pads, both planes
            nc.vector.tensor_copy(
                t[:K, :, 0:2].bitcast(F32R), t[:K, :, 2:3].broadcast_to([K, 2, 2])
            )
            nc.vector.tensor_copy(
                t[:K, :, wpad - 2 : wpad].bitcast(F32R),
                t[:K, :, wpad - 3 : wpad - 2].broadcast_to([K, 2, 2]),
            )

        def emit_pair(pA):
            wins = {}  # (level, j) -> SBUF window tile [K, 2, W+4]

            for l in range(levels - 1):
                Hl = H >> l
                Wl = Hl
                Wc = Wl // 2
                blocks = _blocks(Hl)
                nw = len(blocks)
                cgrid = _grid(Hl // 2)
                # last producer block needed by each coarse window
                last_b = []
                for a0, Kc in cgrid:
                    need = a0 + Kc - 1
                    for b, (_, _, lo, M) in enumerate(blocks):
                        if lo + M > need:
                            last_b.append(b)
                            break
                producers = []

                def get_win(j):
                    if l == 0 and (l, j) not in wins:
                        r0, K, _, _ = blocks[j]
                        t = spool.tile([K, 2, Wl + 4], F32, tag="gwin0", bufs=wbufs[0])
                        nc.sync.dma_start(
                            t[:K, :, 2 : 2 + Wl].bitcast(F32R),
                            xr[r0 : r0 + K, pA : pA + 2, :].bitcast(F32R),
                        )
                        if do_act:
                            pad_fill(t, K, Wl + 4)
                        wins[(l, j)] = t
                    return wins[(l, j)]

                def reduce_block(j):
                    r0, K, m0, M = blocks[j]
                    g = get_win(j)
                    ps = ppool.tile([M, 2, Wc], F32, tag="psr", bufs=2)
                    if not do_pe:
                        pass
                    elif l == 0:
                        for a in (0, 1):
                            for i, d in enumerate(_RTAPS):
                                nc.tensor.matmul(
                                    ps[:, a, :],
                                    wt((l, j, "r", d)),
                                    g[:K, a, d : d + 2 * Wc : 2].bitcast(F32R),
                                    start=(i == 0),
                                    stop=(i == 4),
                                )
                    else:
                        for i, d in enumerate(_RTAPS) if do_pe else ():
                            nc.tensor.matmul(
                                ps[:, :, :],
                                wt((l, j, "r", d)),
                                g[:K, :, d : d + 2 * Wc : 2].bitcast(F32R),
                                start=(i == 0),
                                stop=(i == 4),
                            )
                    gt = spool.tile([M, 2, Wc + 4], F32, tag=f"gp{l}", bufs=4)
                    if do_act:
                        nc.scalar.copy(gt[:M, :, 2 : 2 + Wc].bitcast(F32R), ps[:, :, :])
                        pad_fill(gt, M, Wc + 4)
                    else:
                        nc.sync.dma_start(gt[:M, :, :], g[:M, :, 0 : Wc + 4])
                    producers.append((gt, m0, M))

                def asm_coarse(i):
                    a0, Kc = cgrid[i]
                    t = spool.tile(
                        [Kc, 2, Wc + 4], F32, tag=f"gwin{l + 1}", bufs=wbufs[l + 1]
                    )
                    for src, m0, M in producers:
                        a = max(a0, m0)
                        b = min(a0 + Kc, m0 + M)
                        if a < b:
                            asm_engine().dma_start(
                                t[a - a0 : b - a0, :, :].bitcast(F32R),
                                src[a - m0 : b - m0, :, :].bitcast(F32R),
                            )
                    wins[(l + 1, i)] = t

                def expand_fine(j):
                    r0, K, _, _ = blocks[j]
                    cw = wins[(l + 1, j // 2)]
                    Kc = cgrid[j // 2][1]
                    g = wins.pop((l, j))
                    lpT = spool.tile([K, 2, Wl], F32, tag=f"lpt{l}", bufs=(4 if l else 3))
                    for q, taps in _QTAPS:
                        if l == 0:
                            for a in (0, 1):
                                ps = ppool.tile([K, Wc], F32, tag=f"psq{q}", bufs=2)
                                for i, d in enumerate(taps) if do_pe else ():
                                    nc.tensor.matmul(
                                        ps[:, :],
                                        wt((l, j, q, d)),
                                        cw[:Kc, a, 1 + q + d : 1 + q + d + Wc].bitcast(
                                            F32R
                                        ),
                                        start=(i == 0),
                                        stop=(i == len(taps) - 1),
                                    )
                                if not do_dve:
                                    continue
                                nc.vector.tensor_tensor(
                                    out=lpT[:K, a, q : Wl : 2],
                                    in0=g[:K, a, 2 + q : 2 + q + Wl : 2],
                                    in1=ps[:, :],
                                    op=mybir.AluOpType.subtract,
                                )
                        else:
                            ps = ppool.tile([K, 2, Wc], F32, tag=f"psq{q}", bufs=2)
                            for i, d in enumerate(taps) if do_pe else ():
                                nc.tensor.matmul(
                                    ps[:, :, :],
                                    wt((l, j, q, d)),
                                    cw[:Kc, :, 1 + q + d : 1 + q + d + Wc].bitcast(F32R),
                                    start=(i == 0),
                                    stop=(i == len(taps) - 1),
                                )
                            if not do_dve:
                                continue
                            nc.vector.tensor_tensor(
                                out=lpT[:K, :, q : Wl : 2],
                                in0=g[:K, :, 2 + q : 2 + q + Wl : 2],
                                in1=ps[:, :, :],
                                op=mybir.AluOpType.subtract,
                            )
                    e0 = 124 * j
                    e1 = min(124 * (j + 1), Hl)
                    src = (
                        lpT[e0 - r0 : e1 - r0, :, :]
                        if do_dve
                        else g[e0 - r0 : e1 - r0, :, 0:Wl]
                    )
                    nc.scalar.dma_start(lprs[l][e0:e1, pA : pA + 2, :], src)

                for j in range(nw):
                    reduce_block(j)
                    for i, lb in enumerate(last_b):
                        if lb == j:
                            asm_coarse(i)
                    if j >= 2:
                        expand_fine(j - 2)
                    yield
                for j in range(max(0, nw - 2), nw):
                    expand_fine(j)
                    yield

            # deepest level: lp4 = g4, one window holds it whole per pair
            Hd = H >> (levels - 1)
            gd = wins.pop((levels - 1, 0))
            nc.scalar.dma_start(
                lprs[levels - 1][0:Hd, pA : pA + 2, :], gd[:Hd, :, 2 : 2 + Hd]
            )

        for _rep in range(repeat):
            for pr in range(planes // 2):
                for _ in emit_pair(2 * pr):
                    pass

    nc.compile()
    return nc


_CACHE = {}


def _get_nc(H, planes, levels):
    key = (H, planes, levels)
    if key not in _CACHE:
        _CACHE[key] = build_nc(H, planes, levels)
    return _CACHE[key]


def kernel(im, levels):
    levels = int(np.asarray(levels))
    assert levels == 5
    im = np.ascontiguousarray(np.asarray(im), dtype=np.float32)
    B, C, H, W = im.shape
    assert (B, C, H, W) == (16, 3, 1024, 1024)
    n_cores = 8
    per = B // n_cores
    planes = per * C

    nc = _get_nc(H, planes, levels)
    in_maps = [
        {"x": np.ascontiguousarray(im[i * per : (i + 1) * per].reshape(planes * H, W))}
        for i in range(n_cores)
    ]
    res = run_bass_kernel_spmd(nc, in_maps, list(range(n_cores))).results
    out = []
    for l in range(levels):
        Hl = H >> l
        full = np.concatenate(
            [res[i][f"lp{l}"].reshape(per, C, Hl, Hl) for i in range(n_cores)], axis=0
        )
        out.append(full)
    return tuple(out)
